# revision 2
# baseline (speedup 1.0000x reference)
import sys, os
sys.path.insert(0, "/opt/trn_rl_repo")
import numpy as np
import concourse.bass as bass
import concourse.mybir as mybir
from concourse import tile, bacc
from concourse.bass_utils import run_bass_kernel_spmd

F32 = mybir.dt.float32
AF = mybir.ActivationFunctionType
OP = mybir.AluOpType
NSH = 8
DIMS = [128, 256, 512, 1024]
NSLICES = [64, 32, 16, 4]
HH = [64, 32, 16, 8]
DST = 16
EPS = 1e-5
ACT_F = [AF.Gelu, AF.Gelu, AF.Silu, AF.Silu]
NSTAGE = int(os.environ.get("NSTAGE", "4"))
DEPTH = int(os.environ.get("DEPTH", "99"))  # 1=down only, 2=+gsc, 3=+mamba, 4=+mlp


def _dtr(C):
    return (C + 15) // 16


class B:
    def __init__(self, nc, tc, pools):
        self.nc = nc
        self.tc = tc
        self.p = pools
        self.uid = 0

    def tag(self, s):
        self.uid += 1
        return f"{s}_{self.uid}"


def chunks(L):
    Lc = min(512, L)
    return [(i * Lc, Lc) for i in range(L // Lc)]


def src_ap(src, bi, cb, P=128):
    """AP for 128-channel block cb of sample bi from gform or full dram."""
    kind, g = src
    if kind == "g":
        C8 = g.shape[2]
        m = P // C8
        return g[cb * m:(cb + 1) * m, bi, :, :]
    return g[bi, cb * P:(cb + 1) * P, :]


def load_blocks(b_, src, bi, C, L, tagp):
    nc = b_.nc
    ncb = (C + 127) // 128
    tiles = []
    for cb in range(ncb):
        t = b_.p["acts"].tile([128, L], F32, tag=f"{tagp}_cb{cb}")
        nc.sync.dma_start(t[:], src_ap(src, bi, cb))
        tiles.append(t)
    return tiles


def inorm_act(b_, x, P, L, act=None):
    nc = b_.nc
    sm = b_.p["small"]
    s1 = sm.tile([P, 1], F32, tag="in_s1")
    s2 = sm.tile([P, 1], F32, tag="in_s2")
    sq = b_.p["scratch"].tile([P, L], F32, tag="in_sq")
    nc.vector.tensor_reduce(s1[:], x, mybir.AxisListType.X, OP.add)
    nc.scalar.activation(sq[:], x, AF.Square, accum_out=s2[:])
    mean = sm.tile([P, 1], F32, tag="in_mn")
    nc.vector.tensor_scalar_mul(mean[:], s1[:], 1.0 / L)
    msq = sm.tile([P, 1], F32, tag="in_msq")
    nc.vector.tensor_tensor(msq[:], mean[:], mean[:], OP.mult)
    var = sm.tile([P, 1], F32, tag="in_var")
    nc.vector.scalar_tensor_tensor(var[:], s2[:], 1.0 / L, msq[:], OP.mult, OP.subtract)
    sd = sm.tile([P, 1], F32, tag="in_sd")
    nc.scalar.activation(sd[:], var[:], AF.Sqrt, bias=EPS)
    rstd = sm.tile([P, 1], F32, tag="in_rstd")
    nc.vector.reciprocal(rstd[:], sd[:])
    nc.vector.tensor_scalar(x, x, mean[:], rstd[:], OP.subtract, op1=OP.mult)
    if act is not None:
        nc.scalar.activation(x, x, act)


def conv2d(b_, src, bi, Cin, Co8, H, W, K, S, pad, wT, inorm_in=False):
    nc = b_.nc
    Hp, Wp = H + 2 * pad, W + 2 * pad
    Ho, Wo = (Hp - K) // S + 1, (Wp - K) // S + 1
    Lo = Ho * Wo
    if src[0] == "x3":
        pt = b_.p["acts"].tile([3, Hp * Wp], F32, tag="pad_cb0")
        nc.sync.dma_start(pt[:], src[1][bi])
        inp, ncb, Pin = [pt], 1, 3
    else:
        ncb, Pin = (Cin + 127) // 128, 128
        inp = []
        for cb in range(ncb):
            t = b_.p["acts"].tile([128, Hp * Wp], F32, tag=f"pad_cb{cb}")
            if pad > 0:
                nc.vector.memset(t[:], 0.0)
                dst = t[:].rearrange("p (h w) -> p h w", h=Hp)[:, pad:pad + H, pad:pad + W]
            else:
                dst = t[:]
            nc.sync.dma_start(dst, src_ap(src, bi, cb))
            if inorm_in:
                inorm_act(b_, t[:], 128, Hp * Wp)
            inp.append(t)
    wts = {}
    for t in range(K * K):
        for cb in range(ncb):
            w_ = b_.p["wts"].tile([Pin, Co8], F32, tag=f"w_t{t}_cb{cb}")
            nc.sync.dma_start(w_[:], wT[t, cb * Pin:(cb + 1) * Pin, :])
            wts[(t, cb)] = w_
    out = b_.p["osh"].tile([Co8, Lo], F32, tag="convo")
    rc = max(1, min(512 // Wo, Ho))
    for r0 in range(0, Ho, rc):
        ps = b_.p["ps"].tile([Co8, rc * Wo], F32, tag="conv_ps")
        first = True
        for t in range(K * K):
            dy, dx = t // K, t % K
            for cb in range(ncb):
                v = inp[cb][:].rearrange("p (h w) -> p h w", h=Hp)
                rhs = v[:, dy + r0 * S: dy + r0 * S + (rc - 1) * S + 1: S,
                        dx: dx + (Wo - 1) * S + 1: S]
                nc.tensor.matmul(ps[:], wts[(t, cb)][:], rhs,
                                 start=first, stop=(t == K * K - 1 and cb == ncb - 1))
                first = False
        nc.scalar.activation(out[:, r0 * Wo:(r0 + rc) * Wo], ps[:], AF.Copy)
    return out


def bounce_ag(b_, write_fn, C8, L):
    """write_fn(ib) issues DMAs filling ib (2, C8, L). Returns gform."""
    nc = b_.nc
    ib = b_.p["dram"].tile([2, C8, L], F32, tag=b_.tag("agi"))
    write_fn(ib)
    ob = b_.p["dram"].tile([NSH, 2, C8, L], F32, tag=b_.tag("ago"))
    nc.gpsimd.collective_compute("AllGather", OP.bypass,
                                 replica_groups=[list(range(NSH))],
                                 ins=[ib.opt()], outs=[ob.opt()])
    return ("g", ob)


def gsc(b_, s, src):
    C = DIMS[s]
    H = W = HH[s]
    L = H * W
    C8 = C // NSH
    act = ACT_F[s]
    nc = b_.nc
    wts = {c: b_.wt[f"s{s}_g{c}_wT"] for c in range(1, 5)}

    def phase1(ib):
        for bi in range(2):
            t = conv2d(b_, src, bi, C, C8, H, W, 3, 1, 1, wts[1])
            inorm_act(b_, t[:], C8, L, act)
            nc.sync.dma_start(ib[bi], t[:])
    g1 = bounce_ag(b_, phase1, C8, L)

    def phase2(ib):
        for bi in range(2):
            t = conv2d(b_, g1, bi, C, C8, H, W, 3, 1, 1, wts[2])
            inorm_act(b_, t[:], C8, L, act)
            t3 = conv2d(b_, src, bi, C, C8, H, W, 1, 1, 0, wts[3])
            inorm_act(b_, t3[:], C8, L, act)
            nc.vector.tensor_tensor(t3[:], t3[:], t[:], OP.add)
            nc.sync.dma_start(ib[bi], t3[:])
    g12 = bounce_ag(b_, phase2, C8, L)

    def phase3(ib):
        for bi in range(2):
            t = conv2d(b_, g12, bi, C, C8, H, W, 1, 1, 0, wts[4])
            inorm_act(b_, t[:], C8, L, act)
            nc.sync.dma_start(ib[bi], t[:])
    g4 = bounce_ag(b_, phase3, C8, L)
    # residual add on full tensors (redundant per core), local -> full form
    of = b_.p["dram"].tile([2, C, L], F32, tag=b_.tag("gscf"))
    ncb = C // 128
    for bi in range(2):
        for cb in range(ncb):
            a = b_.p["acts"].tile([128, L], F32, tag="res_a")
            x = b_.p["acts"].tile([128, L], F32, tag="res_x")
            nc.sync.dma_start(a[:], src_ap(g4, bi, cb))
            nc.sync.dma_start(x[:], src_ap(src, bi, cb))
            nc.vector.tensor_tensor(a[:], a[:], x[:], OP.add)
            nc.sync.dma_start(of[bi, cb * 128:(cb + 1) * 128, :], a[:])
    return ("f", of)


def mamba(b_, s, j, src):
    nc = b_.nc
    C = DIMS[s]
    L = HH[s] * HH[s]
    D8 = 2 * C // NSH
    dtr = _dtr(C)
    M = dtr + 2 * DST
    ns = NSLICES[s]
    r = L // ns
    ndm = (D8 + 127) // 128
    ncb = C // 128
    pre = f"s{s}_m{j}"
    CH = chunks(L)
    nj = D8 // 8
    xdbl_i = b_.p["dram"].tile([2, 3, M, L], F32, tag=b_.tag("xdi"))
    xe_sp = b_.p["dram"].tile([2, 3, D8, L], F32, tag=b_.tag("xesp"))
    zs_sp = b_.p["dram"].tile([2, D8, L], F32, tag=b_.tag("zssp"))
    for bi in range(2):
        # LayerNorm over C
        xb = load_blocks(b_, src, bi, C, L, "lnx")
        s1 = b_.p["bc"].tile([1, L], F32, tag="ln_s1")
        s2 = b_.p["bc"].tile([1, L], F32, tag="ln_s2")
        for l0, lc in CH:
            p1 = b_.p["ps"].tile([1, 512], F32, tag="ln_p1")
            p2 = b_.p["ps"].tile([1, 512], F32, tag="ln_p2")
            for cb in range(ncb):
                nc.tensor.matmul(p1[:, :lc], b_.ones_p1[:], xb[cb][:, l0:l0 + lc],
                                 start=(cb == 0), stop=(cb == ncb - 1))
                sq = b_.p["scratch"].tile([128, 512], F32, tag="ln_sq")
                nc.scalar.activation(sq[:, :lc], xb[cb][:, l0:l0 + lc], AF.Square)
                nc.tensor.matmul(p2[:, :lc], b_.ones_p1[:], sq[:, :lc],
                                 start=(cb == 0), stop=(cb == ncb - 1))
            nc.scalar.activation(s1[:, l0:l0 + lc], p1[:, :lc], AF.Copy, scale=1.0 / C)
            nc.scalar.activation(s2[:, l0:l0 + lc], p2[:, :lc], AF.Copy, scale=1.0 / C)
        msq = b_.p["bc"].tile([1, L], F32, tag="ln_msq")
        nc.vector.tensor_tensor(msq[:], s1[:], s1[:], OP.mult)
        var = b_.p["bc"].tile([1, L], F32, tag="ln_var")
        nc.vector.tensor_tensor(var[:], s2[:], msq[:], OP.subtract)
        sd = b_.p["bc"].tile([1, L], F32, tag="ln_sd")
        nc.scalar.activation(sd[:], var[:], AF.Sqrt, bias=EPS)
        rstd = b_.p["bc"].tile([1, L], F32, tag="ln_rstd")
        nc.vector.reciprocal(rstd[:], sd[:])
        mrs = b_.p["bc"].tile([1, L], F32, tag="ln_mrs")
        nc.vector.scalar_tensor_tensor(mrs[:], s1[:], -1.0, rstd[:], OP.mult, OP.mult)
        for cb in range(ncb):
            lnw = b_.p["vecs"].tile([128, 1], F32, tag="lnw")
            lnb = b_.p["vecs"].tile([128, 1], F32, tag="lnb")
            nc.sync.dma_start(lnw[:], b_.wt[pre + "_lnw"][cb * 128:(cb + 1) * 128])
            nc.sync.dma_start(lnb[:], b_.wt[pre + "_lnb"][cb * 128:(cb + 1) * 128])
            for l0, lc in CH:
                pa = b_.p["ps"].tile([128, 512], F32, tag="ln_pa")
                pb = b_.p["ps"].tile([128, 512], F32, tag="ln_pb")
                nc.tensor.matmul(pa[:, :lc], b_.ones_1p[:], rstd[:, l0:l0 + lc], start=True, stop=True)
                nc.tensor.matmul(pb[:, :lc], b_.ones_1p[:], mrs[:, l0:l0 + lc], start=True, stop=True)
                t_ = xb[cb][:, l0:l0 + lc]
                nc.vector.tensor_tensor(t_, t_, pa[:, :lc], OP.mult)
                nc.vector.tensor_tensor(t_, t_, pb[:, :lc], OP.add)
            nc.vector.tensor_scalar(xb[cb][:], xb[cb][:], lnw[:], lnb[:], OP.mult, op1=OP.add)
        # in_proj -> xz_x, xz_z (SBUF, this bi only)
        xz = {}
        for dm in range(ndm):
            dmP = min(128, D8 - dm * 128)
            for part in ("ipxT", "ipzT"):
                ot = b_.p["mper"].tile([dmP, L], F32, tag=f"xz_{part}_{dm}")
                for l0, lc in CH:
                    ps = b_.p["ps"].tile([128, 512], F32, tag="ip_ps")
                    for cb in range(ncb):
                        wt = b_.p["wts"].tile([128, dmP], F32, tag="ip_w")
                        nc.sync.dma_start(wt[:], b_.wt[f"{pre}_{part}"][cb * 128:(cb + 1) * 128,
                                                                        dm * 128:dm * 128 + dmP])
                        nc.tensor.matmul(ps[:dmP, :lc], wt[:], xb[cb][:, l0:l0 + lc],
                                         start=(cb == 0), stop=(cb == ncb - 1))
                    nc.scalar.activation(ot[:, l0:l0 + lc], ps[:dmP, :lc], AF.Copy)
                xz[(part, dm)] = ot
            zst = b_.p["mper"].tile([dmP, L], F32, tag=f"zs_{dm}")
            nc.scalar.activation(zst[:], xz[("ipzT", dm)][:], AF.Silu)
            nc.sync.dma_start(zs_sp[bi, dm * 128:dm * 128 + dmP, :], zst[:])
        # branches: conv1d + silu + x_proj partial
        for ib, br in enumerate("fbs"):
            xe_l = {}
            for dm in range(ndm):
                dmP = min(128, D8 - dm * 128)
                xcp = b_.p["scratch"].tile([dmP, L + 3], F32, tag="xcp")
                nc.vector.memset(xcp[:, 0:3], 0.0)
                sx = xz[("ipxT", dm)][:]
                if br == "b":
                    nc.vector.tensor_copy(xcp[:, 3:], sx[:, ::-1])
                elif br == "s":
                    nc.vector.tensor_copy(xcp[:, 3:].rearrange("p (a k) -> p a k", a=r),
                                          sx.rearrange("p (k a) -> p a k", k=ns))
                else:
                    nc.vector.tensor_copy(xcp[:, 3:], sx)
                cw = b_.p["vecs"].tile([dmP, 4], F32, tag="cw")
                nc.sync.dma_start(cw[:], b_.wt[f"{pre}_{br}_cw"][dm * 128:dm * 128 + dmP, :])
                cbb = b_.p["vecs"].tile([dmP, 1], F32, tag="cbb")
                nc.sync.dma_start(cbb[:], b_.wt[f"{pre}_{br}_cb"][dm * 128:dm * 128 + dmP])
                xe = b_.p["mper"].tile([dmP, L], F32, tag=f"xe_{dm}")
                nc.vector.tensor_scalar_mul(xe[:], xcp[:, 0:L], cw[:, 0:1])
                for k in range(1, 4):
                    nc.vector.scalar_tensor_tensor(xe[:], xcp[:, k:k + L], cw[:, k:k + 1],
                                                   xe[:], OP.mult, OP.add)
                nc.scalar.activation(xe[:], xe[:], AF.Silu, bias=cbb[:])
                nc.sync.dma_start(xe_sp[bi, ib, dm * 128:dm * 128 + dmP, :], xe[:])
                xe_l[dm] = xe
            for l0, lc in CH:
                ps = b_.p["ps"].tile([M, 512], F32, tag="xp_ps")
                for dm in range(ndm):
                    dmP = min(128, D8 - dm * 128)
                    wt = b_.p["wts"].tile([dmP, M], F32, tag="xp_w")
                    nc.sync.dma_start(wt[:], b_.wt[f"{pre}_{br}_xpT"][dm * 128:dm * 128 + dmP, :])
                    nc.tensor.matmul(ps[:, :lc], wt[:], xe_l[dm][:, l0:l0 + lc],
                                     start=(dm == 0), stop=(dm == ndm - 1))
                nc.sync.dma_start(xdbl_i[bi, ib, :, l0:l0 + lc], ps[:, :lc])
    xdbl_o = b_.p["dram"].tile([2, 3, M, L], F32, tag=b_.tag("xdo"))
    nc.gpsimd.collective_compute("AllReduce", OP.add, replica_groups=[list(range(NSH))],
                                 ins=[xdbl_i.opt()], outs=[xdbl_o.opt()])
    # phase 2
    op_i = b_.p["dram"].tile([2, C, L], F32, tag=b_.tag("opi"))
    for bi in range(2):
        ysum = {}
        for ib, br in enumerate("fbs"):
            dtraw = b_.p["scratch"].tile([dtr, L], F32, tag="dtraw")
            nc.sync.dma_start(dtraw[:], xdbl_o[bi, ib, 0:dtr, :])
            B16 = b_.p["scratch"].tile([DST, L], F32, tag="B16")
            nc.sync.dma_start(B16[:], xdbl_o[bi, ib, dtr:dtr + DST, :])
            C16 = b_.p["scratch"].tile([DST, L], F32, tag="C16")
            nc.sync.dma_start(C16[:], xdbl_o[bi, ib, dtr + DST:dtr + 2 * DST, :])
            Brep = b_.p["mper"].tile([128, L], F32, tag="Brep")
            Crep = b_.p["mper"].tile([128, L], F32, tag="Crep")
            for l0, lc in CH:
                pB = b_.p["ps"].tile([128, 512], F32, tag="rep_ps")
                nc.tensor.matmul(pB[:, :lc], b_.sel8n[:], B16[:, l0:l0 + lc], start=True, stop=True)
                nc.scalar.activation(Brep[:, l0:l0 + lc], pB[:, :lc], AF.Copy)
                pC = b_.p["ps"].tile([128, 512], F32, tag="rep_ps")
                nc.tensor.matmul(pC[:, :lc], b_.sel8n[:], C16[:, l0:l0 + lc], start=True, stop=True)
                nc.scalar.activation(Crep[:, l0:l0 + lc], pC[:, :lc], AF.Copy)
            dt_t, dtu_t, xe_t, y1_t = {}, {}, {}, {}
            for dm in range(ndm):
                dmP = min(128, D8 - dm * 128)
                wt = b_.p["wts"].tile([dtr, dmP], F32, tag="dt_w")
                nc.sync.dma_start(wt[:], b_.wt[f"{pre}_{br}_dtT"][:, dm * 128:dm * 128 + dmP])
                dtb = b_.p["vecs"].tile([dmP, 1], F32, tag="dtb")
                nc.sync.dma_start(dtb[:], b_.wt[f"{pre}_{br}_dtb"][dm * 128:dm * 128 + dmP])
                dt_sb = b_.p["mper"].tile([dmP, L], F32, tag=f"dt_{dm}")
                for l0, lc in CH:
                    ps = b_.p["ps"].tile([128, 512], F32, tag="dt_ps")
                    nc.tensor.matmul(ps[:dmP, :lc], wt[:], dtraw[:, l0:l0 + lc], start=True, stop=True)
                    nc.scalar.activation(dt_sb[:, l0:l0 + lc], ps[:dmP, :lc], AF.Softplus, bias=dtb[:])
                xe = b_.p["mper"].tile([dmP, L], F32, tag=f"xe2_{dm}")
                nc.sync.dma_start(xe[:], xe_sp[bi, ib, dm * 128:dm * 128 + dmP, :])
                dtu = b_.p["mper"].tile([dmP, L], F32, tag=f"dtu_{dm}")
                nc.vector.tensor_tensor(dtu[:], dt_sb[:], xe[:], OP.mult)
                dt_t[dm], dtu_t[dm], xe_t[dm] = dt_sb, dtu, xe
                y1_t[dm] = b_.p["mper"].tile([dmP, L], F32, tag=f"y1_{dm}")
            hlast = {}
            for jj in range(nj):
                hlast[jj] = b_.p["vecs"].tile([128, 1], F32, tag=f"hl_{jj}")
                nc.vector.memset(hlast[jj][:], 0.0)
            An_t = {}
            for jj in range(nj):
                An_t[jj] = b_.p["vecs"].tile([128, 1], F32, tag=f"An_{jj}")
                nc.sync.dma_start(An_t[jj][:], b_.wt[f"{pre}_{br}_An"][jj, :])
            Dv_t = {}
            for dm in range(ndm):
                dmP = min(128, D8 - dm * 128)
                Dv_t[dm] = b_.p["vecs"].tile([dmP, 1], F32, tag=f"Dv_{dm}")
                nc.sync.dma_start(Dv_t[dm][:], b_.wt[f"{pre}_{br}_Dv"][dm * 128:dm * 128 + dmP])
            for l0, lc in CH:
                for dm in range(ndm):
                    dmP = min(128, D8 - dm * 128)
                    psy = b_.p["ps"].tile([128, 512], F32, tag="psy")
                    for jj in range(dm * 16, dm * 16 + dmP // 8):
                        ro = (jj * 8) % 128
                        p1 = b_.p["ps"].tile([128, 512], F32, tag="sc_ps1")
                        nc.tensor.matmul(p1[:, :lc], b_.sel16[:], dt_t[dm][ro:ro + 8, l0:l0 + lc],
                                         start=True, stop=True)
                        dA = b_.p["big"].tile([128, 512], F32, tag="dA")
                        nc.scalar.activation(dA[:, :lc], p1[:, :lc], AF.Exp, scale=An_t[jj][:])
                        p2 = b_.p["ps"].tile([128, 512], F32, tag="sc_ps2")
                        nc.tensor.matmul(p2[:, :lc], b_.sel16[:], dtu_t[dm][ro:ro + 8, l0:l0 + lc],
                                         start=True, stop=True)
                        dBx = b_.p["big"].tile([128, 512], F32, tag="dBx")
                        nc.vector.tensor_tensor(dBx[:, :lc], p2[:, :lc], Brep[:, l0:l0 + lc], OP.mult)
                        h = b_.p["big"].tile([128, 512], F32, tag="hh")
                        nc.vector.tensor_tensor_scan(h[:, :lc], dA[:, :lc], dBx[:, :lc],
                                                     hlast[jj][:], OP.mult, OP.add)
                        nc.vector.tensor_copy(hlast[jj][:], h[:, lc - 1:lc])
                        hc = b_.p["big"].tile([128, 512], F32, tag="hC")
                        nc.vector.tensor_tensor(hc[:, :lc], h[:, :lc], Crep[:, l0:l0 + lc], OP.mult)
                        nc.tensor.matmul(psy[ro:ro + 8, :lc], b_.selr8[:], hc[:, :lc],
                                         start=True, stop=True)
                    nc.vector.scalar_tensor_tensor(y1_t[dm][:, l0:l0 + lc], xe_t[dm][:, l0:l0 + lc],
                                                   Dv_t[dm][:], psy[:dmP, :lc], OP.mult, OP.add)
            # gate + accumulate into ysum
            for dm in range(ndm):
                dmP = min(128, D8 - dm * 128)
                zst = b_.p["mper"].tile([dmP, L], F32, tag=f"zs2_{dm}")
                nc.sync.dma_start(zst[:], zs_sp[bi, dm * 128:dm * 128 + dmP, :])
                y1 = y1_t[dm]
                if br == "b":
                    nc.vector.tensor_tensor(y1[:], y1[:], zst[:, ::-1], OP.mult)
                elif br == "s":
                    nc.vector.tensor_tensor(y1[:].rearrange("p (a k) -> p a k", a=r),
                                            y1[:].rearrange("p (a k) -> p a k", a=r),
                                            zst[:].rearrange("p (k a) -> p a k", k=ns), OP.mult)
                else:
                    nc.vector.tensor_tensor(y1[:], y1[:], zst[:], OP.mult)
                if br == "f":
                    ys = b_.p["mper"].tile([dmP, L], F32, tag=f"ysum_{dm}")
                    nc.vector.tensor_copy(ys[:], y1[:])
                    ysum[dm] = ys
                elif br == "b":
                    nc.vector.tensor_tensor(ysum[dm][:], ysum[dm][:], y1[:, ::-1], OP.add)
                else:
                    nc.vector.tensor_tensor(ysum[dm][:].rearrange("p (k a) -> p k a", k=ns),
                                            ysum[dm][:].rearrange("p (k a) -> p k a", k=ns),
                                            y1[:].rearrange("p (a k) -> p k a", a=r), OP.add)
        # out_proj partial
        ncm = C // 128
        for cm in range(ncm):
            for l0, lc in CH:
                ps = b_.p["ps"].tile([128, 512], F32, tag="op_ps")
                for dm in range(ndm):
                    dmP = min(128, D8 - dm * 128)
                    wt = b_.p["wts"].tile([dmP, 128], F32, tag="op_w")
                    nc.sync.dma_start(wt[:], b_.wt[f"{pre}_opT"][dm * 128:dm * 128 + dmP,
                                                                 cm * 128:(cm + 1) * 128])
                    nc.tensor.matmul(ps[:, :lc], wt[:], ysum[dm][:, l0:l0 + lc],
                                     start=(dm == 0), stop=(dm == ndm - 1))
                nc.sync.dma_start(op_i[bi, cm * 128:(cm + 1) * 128, l0:l0 + lc], ps[:, :lc])
    op_o = b_.p["dram"].tile([2, C, L], F32, tag=b_.tag("opo"))
    nc.gpsimd.collective_compute("AllReduce", OP.add, replica_groups=[list(range(NSH))],
                                 ins=[op_i.opt()], outs=[op_o.opt()])
    return ("f", op_o)


def mlp(b_, s, src):
    nc = b_.nc
    C = DIMS[s]
    L = HH[s] * HH[s]
    C28 = 2 * C // NSH
    C8 = C // NSH
    ncb = C // 128
    act = ACT_F[s]
    CH = chunks(L)
    n2m = (C28 + 127) // 128

    def phase1(ib):
        for bi in range(2):
            xb = load_blocks(b_, src, bi, C, L, "mlpx")
            for cb in range(ncb):
                inorm_act(b_, xb[cb][:], 128, L)
            for dm in range(n2m):
                dmP = min(128, C28 - dm * 128)
                f1b = b_.p["vecs"].tile([dmP, 1], F32, tag="f1b")
                nc.sync.dma_start(f1b[:], b_.wt[f"s{s}_f1b"][dm * 128:dm * 128 + dmP])
                ht = b_.p["osh"].tile([dmP, L], F32, tag="mlph")
                for l0, lc in CH:
                    ps = b_.p["ps"].tile([128, 512], F32, tag="f1_ps")
                    for cb in range(ncb):
                        wt = b_.p["wts"].tile([128, dmP], F32, tag="f1_w")
                        nc.sync.dma_start(wt[:], b_.wt[f"s{s}_f1T"][cb * 128:(cb + 1) * 128,
                                                                    dm * 128:dm * 128 + dmP])
                        nc.tensor.matmul(ps[:dmP, :lc], wt[:], xb[cb][:, l0:l0 + lc],
                                         start=(cb == 0), stop=(cb == ncb - 1))
                    nc.scalar.activation(ht[:, l0:l0 + lc], ps[:dmP, :lc], act, bias=f1b[:])
                nc.sync.dma_start(ib[bi, dm * 128:dm * 128 + dmP, :], ht[:])
    gh = bounce_ag(b_, phase1, C28, L)
    n2cb = 2 * C // 128

    def phase2(ib):
        for bi in range(2):
            hb = load_blocks(b_, gh, bi, 2 * C, L, "mlph2")
            f2b = b_.p["vecs"].tile([C8, 1], F32, tag="f2b")
            nc.sync.dma_start(f2b[:], b_.wt[f"s{s}_f2b"][:])
            ot = b_.p["osh"].tile([C8, L], F32, tag="mlpo")
            for l0, lc in CH:
                ps = b_.p["ps"].tile([C8, 512], F32, tag="f2_ps")
                for cb in range(n2cb):
                    wt = b_.p["wts"].tile([128, C8], F32, tag="f2_w")
                    nc.sync.dma_start(wt[:], b_.wt[f"s{s}_f2T"][cb * 128:(cb + 1) * 128, :])
                    nc.tensor.matmul(ps[:, :lc], wt[:], hb[cb][:, l0:l0 + lc],
                                     start=(cb == 0), stop=(cb == n2cb - 1))
                nc.vector.tensor_scalar_add(ot[:, l0:l0 + lc], ps[:, :lc], f2b[:])
            nc.sync.dma_start(ib[bi], ot[:])
    return bounce_ag(b_, phase2, C8, L)


_CACHE = {}


def build():
    if "nc" in _CACHE:
        return _CACHE["nc"]
    nc = bacc.Bacc("TRN2", target_bir_lowering=False, debug=False, num_devices=NSH)
    names = {}

    def par(name, shape):
        names[name] = shape
        return nc.declare_dram_parameter(name, list(shape), F32, isOutput=False).ap()

    wt = {}
    x_in = par("x_in", (2, 3, 128 * 128))
    for s in range(NSTAGE):
        C = DIMS[s]
        Cp = 3 if s == 0 else DIMS[s - 1]
        D8, dtr = 2 * C // NSH, _dtr(C)
        wt[f"s{s}_dwT"] = par(f"s{s}_dwT", (4, Cp, C // NSH))
        wt[f"s{s}_db"] = par(f"s{s}_db", (C // NSH,))
        for c in range(1, 5):
            T = 9 if c <= 2 else 1
            wt[f"s{s}_g{c}_wT"] = par(f"s{s}_g{c}_wT", (T, C, C // NSH))
        for j in range(2):
            p = f"s{s}_m{j}"
            for nm, shp in (("lnw", (C,)), ("lnb", (C,)), ("ipxT", (C, D8)),
                            ("ipzT", (C, D8)), ("opT", (D8, C))):
                wt[f"{p}_{nm}"] = par(f"{p}_{nm}", shp)
            for br in "fbs":
                q = f"{p}_{br}"
                for nm, shp in (("cw", (D8, 4)), ("cb", (D8,)), ("xpT", (D8, dtr + 2 * DST)),
                                ("dtT", (dtr, D8)), ("dtb", (D8,)), ("An", (D8 // 8, 128)),
                                ("Dv", (D8,))):
                    wt[f"{q}_{nm}"] = par(f"{q}_{nm}", shp)
        wt[f"s{s}_f1T"] = par(f"s{s}_f1T", (C, 2 * C // NSH))
        wt[f"s{s}_f1b"] = par(f"s{s}_f1b", (2 * C // NSH,))
        wt[f"s{s}_f2T"] = par(f"s{s}_f2T", (2 * C, C // NSH))
        wt[f"s{s}_f2b"] = par(f"s{s}_f2b", (C // NSH,))
    for cn, sh_ in (("ones_p1", (128, 1)), ("ones_1p", (1, 128)), ("sel16", (8, 128)),
                    ("sel8n", (16, 128)), ("selr8", (128, 8))):
        wt[cn] = par(cn, sh_)
    CL = DIMS[NSTAGE - 1]
    LL = HH[NSTAGE - 1] ** 2
    out = nc.declare_dram_parameter("out", [NSH, 2, CL // NSH, LL], F32, isOutput=True).ap()

    import contextlib
    with tile.TileContext(nc) as tc:
        with contextlib.ExitStack() as est:
            pools = {}
            for pn, bufs, kw in (("acts", 2, {}), ("wts", 2, {}), ("ps", 2, {"space": "PSUM"}),
                                 ("osh", 2, {}), ("small", 2, {}), ("scratch", 2, {}),
                                 ("dram", 1, {"space": "DRAM"}), ("mper", 1, {}),
                                 ("vecs", 2, {}), ("bc", 1, {}), ("big", 2, {})):
                pools[pn] = est.enter_context(tc.tile_pool(name=pn, bufs=bufs, **kw))
            b_ = B(nc, tc, pools)
            b_.wt = wt
            for cn in ("ones_p1", "ones_1p", "sel16", "sel8n", "selr8"):
                t = pools["vecs"].tile(list(names[cn]), F32, tag=cn + "_c")
                nc.sync.dma_start(t[:], wt[cn][:])
                setattr(b_, cn, t)
            cur = ("x3", x_in)
            for s in range(NSTAGE):
                C = DIMS[s]
                Hin = 128 if s == 0 else HH[s - 1]
                L = HH[s] ** 2

                def down(ib, cur=cur, s=s, Hin=Hin, C=C):
                    for bi in range(2):
                        t = conv2d(b_, cur, bi, 3 if s == 0 else DIMS[s - 1], C // NSH,
                                   Hin, Hin, 2, 2, 0, wt[f"s{s}_dwT"], inorm_in=(s > 0))
                        db = pools["vecs"].tile([C // NSH, 1], F32, tag="db")
                        nc.sync.dma_start(db[:], wt[f"s{s}_db"][:])
                        nc.vector.tensor_scalar_add(t[:], t[:], db[:])
                        nc.sync.dma_start(ib[bi], t[:])
                cur = bounce_ag(b_, down, C // NSH, L)
                if DEPTH >= 2:
                    cur = gsc(b_, s, cur)
                if DEPTH >= 3:
                    for j in range(2):
                        cur = mamba(b_, s, j, cur)
                if DEPTH >= 4:
                    cur = mlp(b_, s, cur)
            kind, g = cur
            if kind == "g":
                nc.sync.dma_start(out[:], g[:])
            else:
                nc.sync.dma_start(out[:], g.rearrange("b (i c) l -> i b c l", i=NSH))
    nc.compile()
    _CACHE["nc"] = nc
    return nc


def prep_maps(x, params):
    x = np.ascontiguousarray(np.asarray(x, np.float32).reshape(2, 3, 128 * 128))
    P = params
    maps = [dict(x_in=x) for _ in range(NSH)]
    consts = {
        "ones_p1": np.ones((128, 1), np.float32),
        "ones_1p": np.ones((1, 128), np.float32),
        "sel16": np.repeat(np.eye(8, dtype=np.float32), 16, axis=1).reshape(8, 128),
        "sel8n": np.tile(np.eye(16, dtype=np.float32), (1, 8)).reshape(16, 128),
        "selr8": np.repeat(np.eye(8, dtype=np.float32), 16, axis=0).reshape(128, 8),
    }
    for m in maps:
        m.update({k: v.copy() for k, v in consts.items()})

    def sh(name, arr, axis):
        arr = np.asarray(arr, np.float32)
        per = arr.shape[axis] // NSH
        for i in range(NSH):
            sl = [slice(None)] * arr.ndim
            sl[axis] = slice(i * per, (i + 1) * per)
            maps[i][name] = np.ascontiguousarray(arr[tuple(sl)])

    def rep(name, arr):
        arr = np.ascontiguousarray(np.asarray(arr, np.float32))
        for i in range(NSH):
            maps[i][name] = arr

    for s in range(NSTAGE):
        if s == 0:
            w, bv = P["stem"]["w"], P["stem"]["b"]
        else:
            w, bv = P["down"][s - 1]["w"], P["down"][s - 1]["b"]
        w = np.asarray(w, np.float32)
        wT = np.transpose(w, (2, 3, 1, 0)).reshape(4, w.shape[1], w.shape[0])
        sh(f"s{s}_dwT", wT, 2)
        sh(f"s{s}_db", bv, 0)
        for c in range(1, 5):
            wg = np.asarray(P["gsc"][s][f"c{c}"]["w"], np.float32)
            K = wg.shape[2]
            wT = np.transpose(wg, (2, 3, 1, 0)).reshape(K * K, wg.shape[1], wg.shape[0])
            sh(f"s{s}_g{c}_wT", wT, 2)
        for j in range(2):
            mp = P["stages"][s][j]
            p = f"s{s}_m{j}"
            rep(p + "_lnw", mp["ln_w"])
            rep(p + "_lnb", mp["ln_b"])
            ipw = np.asarray(mp["in_proj_w"], np.float32)
            Di = ipw.shape[0] // 2
            sh(p + "_ipxT", ipw[:Di].T, 1)
            sh(p + "_ipzT", ipw[Di:].T, 1)
            sh(p + "_opT", np.asarray(mp["out_proj_w"], np.float32).T, 0)
            for br in "fbs":
                bp = mp[br]
                q = f"{p}_{br}"
                sh(q + "_cw", np.asarray(bp["conv_w"], np.float32)[:, 0, :], 0)
                sh(q + "_cb", bp["conv_b"], 0)
                sh(q + "_xpT", np.asarray(bp["xproj_w"], np.float32).T, 0)
                sh(q + "_dtT", np.asarray(bp["dtproj_w"], np.float32).T, 1)
                sh(q + "_dtb", bp["dtproj_b"], 0)
                An = -np.exp(np.asarray(bp["A_log"], np.float32))
                D8 = Di // NSH
                for i in range(NSH):
                    maps[i][q + "_An"] = np.ascontiguousarray(
                        An[i * D8:(i + 1) * D8].reshape(D8 // 8, 128))
                sh(q + "_Dv", bp["D"], 0)
        f1 = np.asarray(P["mlp"][s]["fc1"]["w"], np.float32)[:, :, 0, 0]
        f2 = np.asarray(P["mlp"][s]["fc2"]["w"], np.float32)[:, :, 0, 0]
        sh(f"s{s}_f1T", f1.T, 1)
        sh(f"s{s}_f1b", P["mlp"][s]["fc1"]["b"], 0)
        sh(f"s{s}_f2T", f2.T, 1)
        sh(f"s{s}_f2b", P["mlp"][s]["fc2"]["b"], 0)
    return maps


def kernel(x, params):
    nc = build()
    maps = prep_maps(x, params)
    res = run_bass_kernel_spmd(nc, maps, core_ids=list(range(NSH)))
    o = res.results[0]["out"]
    CL = DIMS[NSTAGE - 1]
    Hl = HH[NSTAGE - 1]
    return np.ascontiguousarray(
        np.transpose(o, (1, 0, 2, 3)).reshape(2, CL, Hl, Hl))
